# revision 2
# baseline (speedup 1.0000x reference)
"""DeepSeek-V2 MLA decoder layer (prefill, T=2048) on 8 Trainium2 NeuronCores.

v2.1 layout:
  Stage 1 (token-parallel, 256 tok/core): qkv_a proj + RMSNorms + k_pe rope.
    kv_a+k_pe transposed and AllGathered (fp16, issued early).  q_a kept in
    SBUF; the q b-projection for ALL 32 heads of the local tokens runs on the
    sender (wqbs streamed, one DMA per peer-phase), rope applied sender-side,
    then TWO AllToAlls (head-groups 01 / 23) deliver each core its 4 heads' q
    feature-major fp16 — the second a2a overlaps attention on heads 0-1.
  Stage 2 (head-parallel): kT/v builds between the b-proj phases, causal
    attention in S^T form, softmax normalization via a PE broadcast matmul and
    folded into the PV PSUM->SBUF copy, o_proj interleaved per query group in
    the second head pass.
  DMA queues: SP = weights/out, Act = ag2 path, DVE = a2a path, Pool = colls.
  Host: sum the 8 partial outputs.
"""
import numpy as np

import concourse.bass as bass
import concourse.mybir as mybir
import concourse.tile as tile
from concourse import bacc
from concourse.bass_utils import run_bass_kernel_spmd
from concourse.masks import make_identity

F16 = mybir.dt.float16
F32 = mybir.dt.float32
F8 = mybir.dt.float8e4
F83 = mybir.dt.float8e3
DRM = mybir.MatmulPerfMode.DoubleRow


def dup2(ap):
    """Stride-0 dim1 pair AP (repeat a K-tile twice for DoubleRow)."""
    import concourse.bass as _b
    return _b.AP(tensor=ap.tensor, offset=ap.offset,
                 ap=[list(ap.ap[0]), [0, 2], list(ap.ap[1])])


def pair2(ap, stride):
    """Strided dim1 pair AP from a 2D [128, w] slice."""
    import concourse.bass as _b
    return _b.AP(tensor=ap.tensor, offset=ap.offset,
                 ap=[list(ap.ap[0]), [stride, 2], list(ap.ap[1])])
AX = mybir.AxisListType
AF = mybir.ActivationFunctionType

NCORES = 8
T, HID, H = 2048, 5120, 32
DN, DR, DV, QL, KL = 128, 64, 128, 1536, 512
EPS = 1e-6
THETA = 10000.0
HPC = H // NCORES            # 4 heads per core
TPC = T // NCORES            # 256 tokens per core
CW = QL + KL + DR            # 2112
KVW = KL + DR                # 576 (AG2 payload rows)
PPW = HPC * (DN + DR)        # 768 rows per peer block
PHW = PPW // 2               # 384 rows per peer half (2 heads)
QW = NCORES * PHW            # 3072 rows per a2a buffer
SM_SCALE = float((DN + DR) ** -0.5)
EXP_BIAS = float(-7.0 * np.log(2.0))
NEG = -1e9
QTILES = T // 128            # 16

OUT_SCALE = 1.0 / 1024.0
_PROGRAM_CACHE = {}


def build_program():
    if "nc" in _PROGRAM_CACHE:
        return _PROGRAM_CACHE["nc"]
    nc = bacc.Bacc("TRN2", target_bir_lowering=False, debug=False,
                   num_devices=NCORES)

    # r8 weights/activations: rows (ktile, hi/lo, partition), e4m3
    hT_d = nc.dram_tensor("hT", [128, (HID // 128) * 2 * TPC], F8,
                          kind="ExternalInput").ap()
    wa_d = nc.dram_tensor("wa", [HID * 2, CW], F8, kind="ExternalInput").ap()
    # sender b-proj weights, cols per peer: [h0n|h1n|E01|O01|h2n|h3n|E23|O23]
    wqbs_d = nc.dram_tensor("wqbs", [QL * 2, NCORES * PPW], F8,
                            kind="ExternalInput").ap()
    wkvb_d = nc.dram_tensor("wkvb", [KL, 1024], F16, kind="ExternalInput").ap()
    wo_d = nc.dram_tensor("wo", [HPC * DV * 2, HID], F8,
                          kind="ExternalInput").ap()
    ctok_d = nc.dram_tensor("ctok", [TPC, 32], F16, kind="ExternalInput").ap()
    stok_d = nc.dram_tensor("stok", [TPC, 32], F16, kind="ExternalInput").ap()
    cosq_d = nc.dram_tensor("cosq", [64, TPC], F32, kind="ExternalInput").ap()
    sinq_d = nc.dram_tensor("sinq", [64, TPC], F32, kind="ExternalInput").ap()
    triT_d = nc.dram_tensor("triT", [128, 128], F32, kind="ExternalInput").ap()
    out_d = nc.dram_tensor("out", [T, HID], F16, kind="ExternalOutput").ap()

    with tile.TileContext(nc) as tc:
        with (
            tc.tile_pool(name="const", bufs=1) as cst,
            tc.tile_pool(name="dram", bufs=1, space="DRAM") as dram,
            tc.tile_pool(name="qres", bufs=1) as qres,
            tc.tile_pool(name="kvres", bufs=1) as kvres,
        ):
            ident16 = cst.tile([128, 128], F16, tag="id16")
            make_identity(nc, ident16[:])
            ones16 = cst.tile([128, 1], F16, tag="ones16")
            nc.vector.memset(ones16[:], 1.0)
            onerow = cst.tile([1, 128], F16, tag="onerow")
            nc.vector.memset(onerow[:], 1.0)
            triT_sb = cst.tile([128, 128], F32, tag="triT")
            nc.sync.dma_start(triT_sb[:], triT_d[:])
            ctok_sb = cst.tile([128, 2, 32], F16, tag="ctok")
            nc.sync.dma_start(ctok_sb[:], ctok_d.rearrange("(a p) f -> p a f", p=128))
            stok_sb = cst.tile([128, 2, 32], F16, tag="stok")
            nc.sync.dma_start(stok_sb[:], stok_d.rearrange("(a p) f -> p a f", p=128))
            cosq_sb = cst.tile([64, TPC], F32, tag="cosq")
            nc.sync.dma_start(cosq_sb[:], cosq_d[:])
            sinq_sb = cst.tile([64, TPC], F32, tag="sinq")
            nc.sync.dma_start(sinq_sb[:], sinq_d[:])
            eps_sb = cst.tile([128, 1], F32, tag="eps")
            nc.vector.memset(eps_sb[:], EPS)
            eps20_sb = cst.tile([128, 1], F32, tag="eps20")
            nc.vector.memset(eps20_sb[:], EPS * 1048576.0)
            eps12_sb = cst.tile([128, 1], F32, tag="eps12")
            nc.vector.memset(eps12_sb[:], EPS * 4096.0)
            ebias_sb = cst.tile([128, 1], F32, tag="ebias")
            nc.vector.memset(ebias_sb[:], EXP_BIAS)

            ag2_in = dram.tile([KVW, TPC], F16, tag="ag2in")
            ag2_out = dram.tile([NCORES * KVW, TPC], F16, tag="ag2out")
            a2a_in = [dram.tile([QW, TPC], F83, tag=f"a2ain{p}", name=f"a2ain{p}")
                      for p in range(2)]
            a2a_out = [dram.tile([QW, TPC], F83, tag=f"a2aout{p}",
                                 name=f"a2aout{p}") for p in range(2)]

            # persistent stage-2 tensors
            qTn = [qres.tile([128, T], F83, tag=f"qTn{h}", name=f"qTn{h}")
                   for h in range(HPC)]
            qTpe = [qres.tile([64, T], F83, tag=f"qTpe{h}", name=f"qTpe{h}")
                    for h in range(HPC)]
            kT = [kvres.tile([128, T], F16, tag=f"kT{h}", name=f"kT{h}")
                  for h in range(HPC)]
            kpeT = kvres.tile([64, T], F16, tag="kpeT")
            v_sb = kvres.tile([128, QTILES, HPC * DV], F16, tag="v_sb")

            # ---------------- Stage 1: token-parallel qkv_a + norms + kpe rope
            with tc.tile_pool(name="ph0", bufs=1) as ph0:
              qaT = ph0.tile([128, QL // 128, TPC], F16, tag="qaT")
              qa8H = ph0.tile([128, QL // 128, TPC], F8, tag="qa8H")
              qa8L = ph0.tile([128, QL // 128, TPC], F8, tag="qa8L")
              wkvb_sb = ph0.tile([128, KL // 128, 1024], F16, tag="wkvb")
              kvaT_sb = [ph0.tile([128, T], F16, tag=f"kvaT{cc}",
                                  name=f"kvaT{cc}") for cc in range(KL // 128)]
              with (
                tc.tile_pool(name="ph1", bufs=1) as ph1,
                tc.tile_pool(name="ph1w", bufs=3) as ph1w,
                tc.tile_pool(name="ph1s", bufs=3) as ph1s,
                tc.tile_pool(name="ph1ps", bufs=2, space="PSUM") as ph1ps,
              ):
                hT_sb = ph1.tile([128, HID // 128, 2, TPC], F8, tag="hT")
                hT_r = hT_d.rearrange("p (ko j t) -> p ko j t", j=2, t=TPC)
                for kg in range(4):
                    nc.sync.dma_start(hT_sb[:, kg * 10:(kg + 1) * 10, :, :],
                                      hT_r[:, kg * 10:(kg + 1) * 10, :, :])
                stage = [ph1.tile([128, CW], F16, tag=f"stage{tt}", name=f"stage{tt}")
                         for tt in range(2)]

                def mm_slices(slices):
                    for n0, w in slices:
                        ps = [ph1ps.tile([128, w], F32, tag=f"s1ps{tt}",
                                         name=f"s1ps{tt}") for tt in range(2)]
                        for kg in range(HID // 512):
                            # 4 ktiles x (hi,lo) rows of the r8 wa
                            wa_t = ph1w.tile([128, 8, w], F8, tag="wa_t",
                                             name="wa_t")
                            src = bass.AP(
                                tensor=wa_d.tensor,
                                offset=wa_d.offset + kg * 1024 * CW + n0,
                                ap=[[CW, 128], [128 * CW, 8], [1, w]])
                            nc.sync.dma_start(wa_t[:], src)
                            last = kg == HID // 512 - 1
                            for j in range(4):
                                kc = kg * 4 + j
                                for tt in range(2):
                                    # hH.(wH+wL)
                                    nc.tensor.matmul(
                                        ps[tt][:],
                                        dup2(hT_sb[:, kc, 0,
                                                   tt * 128:(tt + 1) * 128]),
                                        wa_t[:, 2 * j:2 * j + 2, :],
                                        start=(kc == 0), stop=False,
                                        perf_mode=DRM)
                            for j2 in (0, 2):
                                kc = kg * 4 + j2
                                for tt in range(2):
                                    # hL.wH over ktile pairs
                                    nc.tensor.matmul(
                                        ps[tt][:],
                                        hT_sb[:, kc:kc + 2, 1,
                                              tt * 128:(tt + 1) * 128],
                                        pair2(wa_t[:, 2 * j2, :], 2 * w),
                                        start=False,
                                        stop=(last and j2 == 2),
                                        perf_mode=DRM)
                        for tt in range(2):
                            nc.scalar.copy(stage[tt][:, n0:n0 + w], ps[tt][:])

                # --- kv + pe first (feeds the AllGather); ag2 path on Act queue
                mm_slices([(QL, KL), (QL + KL, DR)])
                for tt in range(2):
                    sums = ph1s.tile([128, 4], F32, tag="s1sums")
                    dump = ph1s.tile([128, 512], F32, tag="s1dump")
                    nc.scalar.activation(dump[:], stage[tt][:, QL:QL + KL],
                                         AF.Square, accum_out=sums[:, 3:4])
                    rkv = ph1s.tile([128, 1], F32, tag="rkv")
                    # sums carry the 1024^2 input scale; rkv absorbs 1/1024
                    nc.scalar.activation(rkv[:], sums[:, 3:4], AF.Sqrt,
                                         bias=eps20_sb[:], scale=1.0 / KL)
                    nc.vector.reciprocal(rkv[:], rkv[:])
                    kva16 = ph1.tile([128, KL], F16, tag=f"kva16_{tt}",
                                     name=f"kva16_{tt}")
                    nc.scalar.activation(kva16[:], stage[tt][:, QL:QL + KL],
                                         AF.Copy, scale=rkv[:])
                    # k_pe rope (host permuted cols to [E32|O32])
                    kpe16 = ph1.tile([128, 64], F16, tag=f"kpe16_{tt}",
                                     name=f"kpe16_{tt}")
                    pe = stage[tt][:, QL + KL:CW]
                    ct, st = ctok_sb[:, tt, :], stok_sb[:, tt, :]
                    t1 = ph1s.tile([128, 32], F32, tag="rt1")
                    t2 = ph1s.tile([128, 32], F32, tag="rt2")
                    nc.vector.tensor_mul(t1[:], pe[:, 0:32], ct)
                    nc.vector.tensor_mul(t2[:], pe[:, 32:64], st)
                    nc.vector.tensor_sub(kpe16[:, 0:32], t1[:], t2[:])
                    t3 = ph1s.tile([128, 32], F32, tag="rt3")
                    t4 = ph1s.tile([128, 32], F32, tag="rt4")
                    nc.vector.tensor_mul(t3[:], pe[:, 32:64], ct)
                    nc.vector.tensor_mul(t4[:], pe[:, 0:32], st)
                    nc.vector.tensor_add(kpe16[:, 32:64], t3[:], t4[:])

                    # transpose kva16 + kpe16 into one staging tile, 1 DMA
                    agst = ph1s.tile([128, 5, 128], F16, tag="agst")
                    for b in range(4):
                        tp = ph1ps.tile([128, 128], F16, tag="s1tp", name="s1tp")
                        nc.tensor.transpose(tp[:], kva16[:, b * 128:(b + 1) * 128],
                                            ident16[:])
                        if b % 2 == 0:
                            nc.vector.tensor_copy(agst[:, b, :], tp[:])
                        else:
                            nc.scalar.copy(agst[:, b, :], tp[:])
                    tp2 = ph1ps.tile([64, 128], F16, tag="s1tp2")
                    nc.tensor.transpose(tp2[:], kpe16[:], ident16[:])
                    nc.vector.tensor_copy(agst[0:64, 4, :], tp2[:])
                    dst = bass.AP(
                        tensor=ag2_in.tensor,
                        offset=ag2_in.offset + tt * 128,
                        ap=[[TPC, 128], [128 * TPC, 4], [1, 128]])
                    nc.scalar.dma_start(dst, agst[:, 0:4, :])
                    dstp = bass.AP(
                        tensor=ag2_in.tensor,
                        offset=ag2_in.offset + KL * TPC + tt * 128,
                        ap=[[TPC, 64], [1, 128]])
                    nc.scalar.dma_start(dstp, agst[0:64, 4, :])

                nc.gpsimd.collective_compute(
                    "AllGather", mybir.AluOpType.bypass,
                    ins=[ag2_in.opt()], outs=[ag2_out.opt()],
                    replica_groups=[list(range(NCORES))])

                # gathered kv_a loads on the Pool queue (only collective-gated
                # DMAs live there, so the wait can't head-block other streams)
                for cc in range(KL // 128):
                    src = bass.AP(tensor=ag2_out.tensor,
                                  offset=ag2_out.offset + cc * 128 * TPC,
                                  ap=[[TPC, 128], [KVW * TPC, NCORES], [1, TPC]])
                    nc.gpsimd.dma_start(
                        kvaT_sb[cc][:].rearrange("p (r t) -> p r t", r=NCORES),
                        src)
                src = bass.AP(tensor=ag2_out.tensor,
                              offset=ag2_out.offset + KL * TPC,
                              ap=[[TPC, 64], [KVW * TPC, NCORES], [1, TPC]])
                nc.gpsimd.dma_start(
                    kpeT[:].rearrange("p (r t) -> p r t", r=NCORES), src)
                # prefetch kv b-proj weights on the Act queue (runs during AG2)
                nc.scalar.dma_start(wkvb_sb[:],
                                    wkvb_d.rearrange("(ko p) c -> p ko c", p=128))

                # --- q part: rmsnorm then transpose into SBUF (feature-major)
                mm_slices([(0, 512), (512, 512), (1024, 512)])
                for tt in range(2):
                    sums = ph1s.tile([128, 4], F32, tag="s1sums")
                    dump = ph1s.tile([128, 512], F32, tag="s1dump")
                    for i in range(3):
                        nc.scalar.activation(dump[:],
                                             stage[tt][:, i * 512:(i + 1) * 512],
                                             AF.Square, accum_out=sums[:, i:i + 1])
                    qs = ph1s.tile([128, 1], F32, tag="qs")
                    nc.vector.reduce_sum(qs[:], sums[:, 0:3], axis=AX.X)
                    rq = ph1s.tile([128, 1], F32, tag="rq")
                    nc.scalar.activation(rq[:], qs[:], AF.Sqrt, bias=eps12_sb[:],
                                         scale=1.0 / (QL * 256.0))
                    nc.vector.reciprocal(rq[:], rq[:])
                    qa16 = ph1.tile([128, QL], F16, tag=f"qa16_{tt}",
                                    name=f"qa16_{tt}")
                    for i in range(3):
                        nc.scalar.activation(qa16[:, i * 512:(i + 1) * 512],
                                             stage[tt][:, i * 512:(i + 1) * 512],
                                             AF.Copy, scale=rq[:])
                    for b in range(QL // 128):
                        tp = ph1ps.tile([128, 128], F16, tag="s1tp", name="s1tp")
                        nc.tensor.transpose(tp[:], qa16[:, b * 128:(b + 1) * 128],
                                            ident16[:])
                        if b % 2 == 0:
                            nc.vector.tensor_copy(
                                qaT[:, b, tt * 128:(tt + 1) * 128], tp[:])
                        else:
                            nc.scalar.copy(
                                qaT[:, b, tt * 128:(tt + 1) * 128], tp[:])

              # --- sender-side q b-projection, phase-major (2 heads per phase)
              # phase ph of peer d: cols d*768 + ph*384, rows [hn,hn,E|O]
              def bp_peer(d, bpw, bps, bpps):
                  wq_t2 = bpw.tile([128, 2 * (QL // 128), PPW], F8,
                                   tag="wq_t", name="wq_t")
                  src = bass.AP(
                      tensor=wqbs_d.tensor,
                      offset=wqbs_d.offset + d * PPW,
                      ap=[[NCORES * PPW, 128], [128 * NCORES * PPW,
                                                2 * (QL // 128)], [1, PPW]])
                  nc.sync.dma_start(wq_t2[:], src)
                  for ph in range(2):
                      psm = [bpps.tile([128, TPC], F32, tag=f"bpps{m}",
                                       name=f"bpps{m}") for m in range(3)]
                      for cc in range(QL // 128):
                          for m in range(3):
                              # (wH,wL).qH
                              nc.tensor.matmul(
                                  psm[m][:],
                                  wq_t2[:, 2 * cc:2 * cc + 2,
                                        ph * PHW + m * 128:
                                        ph * PHW + (m + 1) * 128],
                                  dup2(qa8H[:, cc, :]), start=(cc == 0),
                                  stop=False, perf_mode=DRM)
                      for cc2 in range(0, QL // 128, 2):
                          for m in range(3):
                              # wH.qL over ktile pairs
                              nc.tensor.matmul(
                                  psm[m][:],
                                  pair2(wq_t2[:, 2 * cc2,
                                              ph * PHW + m * 128:
                                              ph * PHW + (m + 1) * 128],
                                        2 * PPW),
                                  qa8L[:, cc2:cc2 + 2, :], start=False,
                                  stop=(cc2 == QL // 128 - 2), perf_mode=DRM)
                      ob = bps.tile([128, 3, TPC], F83, tag="ob", name="ob")
                      nc.scalar.activation(ob[:, 0, :], psm[0][:], AF.Copy,
                                           scale=1.0 / 256.0)
                      nc.scalar.activation(ob[:, 1, :], psm[1][:], AF.Copy,
                                           scale=1.0 / 256.0)
                      # rope on the EO tile: rows 0:64 = E, 64:128 = O
                      t1 = bps.tile([64, TPC], F32, tag="bpt1")
                      t2 = bps.tile([64, TPC], F32, tag="bpt2")
                      nc.vector.tensor_mul(t1[:], psm[2][0:64, :], cosq_sb[:])
                      nc.vector.tensor_mul(t2[:], psm[2][64:128, :], sinq_sb[:])
                      nc.vector.tensor_sub(ob[0:64, 2, :], t1[:], t2[:])
                      t3 = bps.tile([64, TPC], F32, tag="bpt3")
                      t4 = bps.tile([64, TPC], F32, tag="bpt4")
                      nc.vector.tensor_mul(t3[:], psm[2][64:128, :], cosq_sb[:])
                      nc.vector.tensor_mul(t4[:], psm[2][0:64, :], sinq_sb[:])
                      nc.vector.tensor_add(ob[64:128, 2, :], t3[:], t4[:])
                      dst = bass.AP(
                          tensor=a2a_in[ph].tensor,
                          offset=a2a_in[ph].offset + d * PHW * TPC,
                          ap=[[TPC, 128], [128 * TPC, 3], [1, TPC]])
                      nc.gpsimd.dma_start(dst, ob[:])

              # quantize q_a (already x16) to hi/lo e4m3
              nc.scalar.activation(qa8H[:], qaT[:], AF.Copy)
              nc.vector.tensor_sub(qa8L[:], qaT[:], qa8H[:])

              def load_q(p):
                  # q loads: head h lives in a2a_out[h//2]; per-peer block:
                  # [hn(0) hn(128) E(256+32hh) O(320+32hh)]
                  for hh in range(2):
                      h = 2 * p + hh
                      buf = a2a_out[p]
                      src = bass.AP(tensor=buf.tensor,
                                    offset=buf.offset + hh * 128 * TPC,
                                    ap=[[TPC, 128], [PHW * TPC, NCORES],
                                        [1, TPC]])
                      nc.gpsimd.dma_start(
                          qTn[h][:].rearrange("p (r t) -> p r t", r=NCORES),
                          src)
                      srcE = bass.AP(tensor=buf.tensor,
                                     offset=buf.offset + (256 + hh * 32) * TPC,
                                     ap=[[TPC, 32], [PHW * TPC, NCORES],
                                         [1, TPC]])
                      nc.gpsimd.dma_start(
                          qTpe[h][0:32, :].rearrange("p (r t) -> p r t",
                                                     r=NCORES), srcE)
                      srcO = bass.AP(tensor=buf.tensor,
                                     offset=buf.offset + (320 + hh * 32) * TPC,
                                     ap=[[TPC, 32], [PHW * TPC, NCORES],
                                         [1, TPC]])
                      nc.gpsimd.dma_start(
                          qTpe[h][32:64, :].rearrange("p (r t) -> p r t",
                                                      r=NCORES), srcO)

              with (
                    tc.tile_pool(name="bpw", bufs=2) as bpw,
                    tc.tile_pool(name="bps", bufs=2) as bps,
                    tc.tile_pool(name="bpps", bufs=2, space="PSUM") as bpps,
              ):
                  for d in range(NCORES):
                      bp_peer(d, bpw, bps, bpps)
                  nc.gpsimd.collective_compute(
                      "AllToAll", mybir.AluOpType.bypass,
                      ins=[a2a_in[0].opt()], outs=[a2a_out[0].opt()],
                      replica_groups=[list(range(NCORES))])
                  nc.gpsimd.collective_compute(
                      "AllToAll", mybir.AluOpType.bypass,
                      ins=[a2a_in[1].opt()], outs=[a2a_out[1].opt()],
                      replica_groups=[list(range(NCORES))])
                  load_q(0)
                  load_q(1)

                  # ---- kv build from gathered kv_a
                  with (
                      tc.tile_pool(name="kvps", bufs=2, space="PSUM") as kvps,
                  ):
                    for h in range(HPC):
                        for n4 in range(4):
                            pk = kvps.tile([128, 512], F32, tag="kvp")
                            for cc in range(KL // 128):
                                nc.tensor.matmul(
                                    pk[:],
                                    wkvb_sb[:, cc, h * 128:(h + 1) * 128],
                                    kvaT_sb[cc][:, bass.ts(n4, 512)],
                                    start=(cc == 0),
                                    stop=(cc == KL // 128 - 1))
                            if n4 % 2 == 0:
                                nc.scalar.copy(kT[h][:, bass.ts(n4, 512)], pk[:])
                            else:
                                nc.vector.tensor_copy(
                                    kT[h][:, bass.ts(n4, 512)], pk[:])

                        for tt16 in range(QTILES):
                            pv = kvps.tile([128, 512], F32, tag="kvp")
                            for cc in range(KL // 128):
                                nc.tensor.matmul(
                                    pv[:],
                                    kvaT_sb[cc][:, tt16 * 128:(tt16 + 1) * 128],
                                    wkvb_sb[:, cc, 512:1024],
                                    start=(cc == 0), stop=(cc == KL // 128 - 1))
                            if tt16 % 2 == 0:
                                nc.scalar.copy(v_sb[:, tt16, :], pv[:])
                            else:
                                nc.vector.tensor_copy(v_sb[:, tt16, :], pv[:])


            # ---------------- Stage 2b: attention; heads 0-1 first (pass 1),
            # heads 2-3 + interleaved o_proj per query group (pass 2)
            with (
                tc.tile_pool(name="atw", bufs=2) as atw,
                tc.tile_pool(name="atp", bufs=2) as atp,
                tc.tile_pool(name="atps", bufs=2, space="PSUM") as atps,
                tc.tile_pool(name="atrs", bufs=1, space="PSUM") as atrs,
                tc.tile_pool(name="atrb", bufs=1, space="PSUM") as atrb,
                tc.tile_pool(name="atpsA", bufs=2, space="PSUM") as atpsA,
                tc.tile_pool(name="ops", bufs=2, space="PSUM") as ops,
                tc.tile_pool(name="os", bufs=4) as osb_pool,
                tc.tile_pool(name="atn", bufs=1) as atn,
            ):
                attn8 = atn.tile([128, HPC, 2, T], F8, tag="attn8")
                wo_sb = atn.tile([128, HPC, 2, HID], F8, tag="wo_sb")
                nc.sync.dma_start(wo_sb[:],
                                  wo_d.rearrange("(c j p) n -> p c j n",
                                                 j=2, p=128))

                def attend(g, h):
                    nsc = 4 * g + 4
                    PT_g = atw.tile([128, QTILES, 512], F16, tag="PTg")
                    rs_ps = atrs.tile([1, 512], F32, tag="rsps")
                    for sc in range(nsc):
                        kk = sc - 4 * g
                        v0 = 128 * kk if kk >= 0 else 0
                        pS = atps.tile([128, 512], F32, tag="Sps")
                        q0 = g * 512 + v0
                        nc.tensor.matmul(pS[:, v0:512],
                                         kT[h][:, sc * 128:(sc + 1) * 128],
                                         qTn[h][:, q0:(g + 1) * 512],
                                         start=True, stop=False)
                        nc.tensor.matmul(pS[:, v0:512],
                                         kpeT[:, sc * 128:(sc + 1) * 128],
                                         qTpe[h][:, q0:(g + 1) * 512],
                                         start=False, stop=True)
                        if kk >= 0:
                            nc.vector.tensor_add(pS[:, v0:v0 + 128],
                                                 pS[:, v0:v0 + 128], triT_sb[:])
                        nc.scalar.activation(PT_g[:, sc, v0:512],
                                             pS[:, v0:512], AF.Exp,
                                             bias=ebias_sb[:], scale=SM_SCALE / 2.0)
                    # rowsums in a second pass so exp latency never blocks the
                    # in-order PE queue
                    for sc in range(nsc):
                        kk = sc - 4 * g
                        v0 = 128 * kk if kk >= 0 else 0
                        nc.tensor.matmul(rs_ps[:, v0:512], ones16[:],
                                         PT_g[:, sc, v0:512],
                                         start=(sc == 0), stop=(sc == nsc - 1))
                    # reciprocal + broadcast to 128 partitions via PE matmul
                    rec32 = atp.tile([1, 512], F32, tag="rec32")
                    nc.vector.reciprocal(rec32[:], rs_ps[:])
                    rec16 = atp.tile([1, 512], F16, tag="rec16")
                    nc.scalar.activation(rec16[:], rec32[:], AF.Copy,
                                         scale=16.0)
                    rrec_ps = atrb.tile([128, 512], F32, tag="rrecps")
                    nc.tensor.matmul(rrec_ps[:], onerow[:], rec16[:],
                                     start=True, stop=True)
                    rrec = atp.tile([128, 512], F16, tag="rrec")
                    nc.scalar.copy(rrec[:], rrec_ps[:])
                    # PV with normalization folded into the PSUM->SBUF copy;
                    # results quantized to hi/lo e4m3 (16x scale via rrec)
                    tmp16 = atp.tile([128, 512], F16, tag="tmp16")
                    for qq in range(4):
                        qt = 4 * g + qq
                        pA = atpsA.tile([128, 128], F32, tag="Aps")
                        for sc in range(qt + 1):
                            nc.tensor.matmul(pA[:],
                                             v_sb[:, sc, h * 128:(h + 1) * 128],
                                             PT_g[:, sc, qq * 128:(qq + 1) * 128],
                                             start=(sc == 0), stop=(sc == qt))
                        nc.vector.tensor_mul(
                            tmp16[:, qq * 128:(qq + 1) * 128], pA[:],
                            rrec[:, qq * 128:(qq + 1) * 128])
                    gsl = bass.ts(g, 512)
                    nc.scalar.activation(attn8[:, h, 0, gsl], tmp16[:], AF.Copy)
                    nc.vector.tensor_sub(attn8[:, h, 1, gsl], tmp16[:],
                                         attn8[:, h, 0, gsl])

                for g in range(4):
                    attend(g, 0)
                    attend(g, 1)
                for g in range(4):
                    attend(g, 2)
                    attend(g, 3)
                    # o_proj for this query group
                    for n10 in range(10):
                        osb = osb_pool.tile([128, 4, 512], F16, tag="osb")
                        for m in range(4):
                            qt = 4 * g + m
                            po = ops.tile([128, 512], F32, tag="ops")
                            for cc in range(HPC):
                                nc.tensor.matmul(
                                    po[:],
                                    attn8[:, cc, 0:2, qt * 128:(qt + 1) * 128],
                                    dup2(wo_sb[:, cc, 0,
                                               n10 * 512:(n10 + 1) * 512]),
                                    start=(cc == 0), stop=False, perf_mode=DRM)
                            for cc2 in (0, 2):
                                nc.tensor.matmul(
                                    po[:],
                                    attn8[:, cc2:cc2 + 2, 0,
                                          qt * 128:(qt + 1) * 128],
                                    wo_sb[:, cc2:cc2 + 2, 1,
                                          n10 * 512:(n10 + 1) * 512],
                                    start=False, stop=(cc2 == 2),
                                    perf_mode=DRM)
                            if m % 2 == 0:
                                nc.scalar.copy(osb[:, m, :], po[:])
                            else:
                                nc.vector.tensor_copy(osb[:, m, :], po[:])
                        dst = bass.AP(
                            tensor=out_d.tensor,
                            offset=out_d.offset + (g * 512) * HID + n10 * 512,
                            ap=[[HID, 128], [128 * HID, 4], [1, 512]])
                        nc.sync.dma_start(dst, osb[:])

    nc.compile()
    _PROGRAM_CACHE["nc"] = nc
    return nc


def _r8(x, scale):
    """Split scale*x into hi/lo e4m3 planes: returns (hi, lo) float arrays."""
    from ml_dtypes import float8_e4m3 as _f8
    xs = np.clip(np.asarray(x, np.float32) * scale, -224.0, 224.0)
    hi = xs.astype(_f8)
    lo = (xs - hi.astype(np.float32)).astype(_f8)
    return hi, lo


def _r8_rows(x, scale, ktile=128):
    """[K, N] -> [(K/ktile), 2, ktile, N] hi/lo-interleaved, flattened rows."""
    hi, lo = _r8(x, scale)
    K, N = hi.shape
    nk = K // ktile
    arr = np.empty((nk, 2, ktile, N), hi.dtype)
    arr[:, 0] = hi.reshape(nk, ktile, N)
    arr[:, 1] = lo.reshape(nk, ktile, N)
    return np.ascontiguousarray(arr.reshape(2 * K, N))


def _r8_pm(x, scale, ktile=128):
    """[K, N] -> partition-major [ktile, (K/ktile)*2*N] hi/lo r8 planes."""
    hi, lo = _r8(x, scale)
    K, N = hi.shape
    nk = K // ktile
    arr = np.stack([hi.reshape(nk, ktile, N), lo.reshape(nk, ktile, N)],
                   axis=2)            # [nk, ktile, 2, N]
    arr = arr.transpose(1, 0, 2, 3)   # [ktile, nk, 2, N]
    return np.ascontiguousarray(arr.reshape(ktile, nk * 2 * N))


def _host_prep(inputs):
    pos = np.asarray(inputs["positions"]).astype(np.float32)
    inv_freq = 1.0 / (THETA ** (np.arange(0, DR, 2, dtype=np.float32) / DR))
    freqs = pos[:, None] * inv_freq
    cos, sin = np.cos(freqs), np.sin(freqs)

    eo = np.concatenate([np.arange(0, DR, 2), np.arange(1, DR, 2)])
    w_qkv_a = np.asarray(inputs["w_qkv_a"], np.float32).copy()
    w_qkv_a[:, QL + KL:] = w_qkv_a[:, QL + KL:][:, eo]
    w_q_b = np.asarray(inputs["w_q_b"], np.float32) * np.asarray(
        inputs["q_a_ln_w"], np.float32)[:, None]
    w_kv_b = np.asarray(inputs["w_kv_b"], np.float32) * np.asarray(
        inputs["kv_a_ln_w"], np.float32)[:, None]
    w_o = np.asarray(inputs["w_o"], np.float32)
    hidT = np.ascontiguousarray(np.asarray(inputs["hidden_states"], np.float32).T)

    wa8 = _r8_rows(w_qkv_a, 64.0)
    triT = np.tril(np.full((128, 128), NEG, np.float32), -1)

    # sender b-proj weights: per peer d, per phase: [h0n|h1n|E01|O01]
    blocks = []
    for d in range(NCORES):
        for ph in range(2):
            hs = [HPC * d + 2 * ph, HPC * d + 2 * ph + 1]
            cols = [w_q_b[:, h * 192:h * 192 + DN] for h in hs]
            cols.append(np.concatenate(
                [w_q_b[:, h * 192 + DN:(h + 1) * 192][:, eo[:32]] for h in hs],
                axis=1))
            cols.append(np.concatenate(
                [w_q_b[:, h * 192 + DN:(h + 1) * 192][:, eo[32:]] for h in hs],
                axis=1))
            blocks.append(np.concatenate(cols, axis=1))
    wqbs = _r8_rows(np.concatenate(blocks, axis=1), 32.0)

    in_maps = []
    for c in range(NCORES):
        hs = [HPC * c + i for i in range(HPC)]
        kcols = np.concatenate(
            [w_kv_b[:, h * 256:h * 256 + DN] for h in hs], axis=1)
        vcols = np.concatenate(
            [w_kv_b[:, h * 256 + DN:(h + 1) * 256] for h in hs], axis=1)
        wkvb_c = np.concatenate([kcols, vcols], axis=1)
        wo_c = np.concatenate([w_o[h * DV:(h + 1) * DV, :] for h in hs], axis=0)
        sl = slice(c * TPC, (c + 1) * TPC)
        in_maps.append({
            "hT": _r8_pm(np.ascontiguousarray(hidT[:, sl]), 16.0),
            "wa": wa8,
            "wqbs": wqbs,
            "wkvb": np.ascontiguousarray(wkvb_c).astype(np.float16),
            "wo": _r8_rows(wo_c, 64.0),
            "ctok": np.ascontiguousarray(cos[sl] / 1024.0).astype(np.float16),
            "stok": np.ascontiguousarray(sin[sl] / 1024.0).astype(np.float16),
            "cosq": np.ascontiguousarray(np.tile(cos[sl].T, (2, 1)) / 256.0).astype(np.float32),
            "sinq": np.ascontiguousarray(np.tile(sin[sl].T, (2, 1)) / 256.0).astype(np.float32),
            "triT": triT,
        })
    return in_maps


def kernel(**inputs) -> np.ndarray:
    nc = build_program()
    in_maps = _host_prep(inputs)
    res = run_bass_kernel_spmd(nc, in_maps, core_ids=list(range(NCORES)))
    out = np.zeros((T, HID), np.float32)
    for r in res.results:
        out += r["out"].astype(np.float32)
    return out * OUT_SCALE


if __name__ == "__main__":
    build_program()
    print("program built ok")
